# revision 20
# baseline (speedup 1.0000x reference)
"""Trainium2 Bass kernel for nn_BCNLayer (locally-connected 7x7 lattice layer + sigmoid).

Math: y[i,j,b] = sigmoid( sum_{dy,dx in [-3,3]} w[dy+3,dx+3][(i-dy)*W + (j-dx)]
                          * x[(i-dy)*W + (j-dx), b] )   (zero outside lattice)

Strategy:
  - 8-way shard over lattice rows (H=128 -> 16 dest rows/core, 22 source rows
    with 3-row halos, zero-padded at the edges).
  - For one dest row i and source-row offset d (7 of them), the contribution is
    a banded 128x128 matrix (band +-3 over lattice columns) applied to the
    source row's [128 cols x B batch] slab:  out[jd, b] += sum_js
    Wband[js, jd] * x[js, b].  That is exactly nc.tensor.matmul(psum, lhsT=Wband,
    rhs=xrow) accumulated over the 7 source rows.
  - Banded matrices are prebuilt on the host (numpy) and DMA'd in; HW executes
    pure DMA + matmul + sigmoid.
"""

import os

import numpy as np

H = 128
W = 128
HW = H * W
B = 1024
NCORES = 8
T = H // NCORES  # dest rows per core = 16
S = T + 6        # source rows per core (halo 3 each side) = 22
BC = 512         # batch chunk (fp32 moving-operand max N)
NB = B // BC     # chunks = 2

# dtype mode for the matmul: "f32" (exact, 4 cyc/row) or "f32r" (fast, 1 cyc/row)
MM_MODE = os.environ.get("KERNEL_MM_MODE", "f32r")

_cache: dict = {}

# filled by the last kernel() call when KERNEL_TRACE=1
last_exec_time_ns = None
last_results = None


def _build_program(mode: str):
    from contextlib import ExitStack

    import concourse.bacc as bacc
    import concourse.mybir as mybir
    import concourse.tile as tile

    nc = bacc.Bacc(
        "TRN2", target_bir_lowering=False, debug=False, num_devices=NCORES
    )
    mm_dt = {"f32": mybir.dt.float32, "f32r": mybir.dt.float32r}[mode]
    xs = nc.dram_tensor("xs", [S, 128, B], mm_dt, kind="ExternalInput").ap()
    wb = nc.dram_tensor(
        "wb", [T * 7, 128, 128], mm_dt, kind="ExternalInput"
    ).ap()
    y = nc.dram_tensor(
        "y", [T, 128, B], mybir.dt.float32, kind="ExternalOutput"
    ).ap()

    from concourse.tile_rust import add_dep_helper

    with tile.TileContext(nc) as tc, ExitStack() as ctx:
        xpool = ctx.enter_context(tc.tile_pool(name="x", bufs=1))
        wpool = ctx.enter_context(tc.tile_pool(name="w", bufs=1))
        ppool = ctx.enter_context(tc.tile_pool(name="ps", bufs=6, space="PSUM"))
        opool = ctx.enter_context(tc.tile_pool(name="o", bufs=6))

        xt = xpool.tile([128, S * B], mm_dt, tag="xslab")
        wt = wpool.tile([128, T * 7 * 128], mm_dt, tag="wslab")

        # Warm the sigmoid ACT table during the load phase (it otherwise loads
        # lazily right before the first real sigmoid, stalling the pipeline).
        warm = opool.tile([128, 1], mybir.dt.float32, tag="warm")
        nc.vector.memset(warm[:], 0.0)
        nc.scalar.activation(warm[:], warm[:], mybir.ActivationFunctionType.Sigmoid)

        # x loads stream on the SP HWDGE ring in batch-chunk-major order so
        # compute (chunk-outer loops below) can start after ~1/4 of the data.
        # wb loads go on the SWDGE (gpsimd) ring, which runs concurrently.
        xt3 = xt[:].rearrange("p (s b) -> p s b", s=S)

        def load_x(lo, hi, c):
            nc.sync.dma_start(
                out=xt3[:, lo:hi, c * BC : (c + 1) * BC],
                in_=xs[lo:hi, :, c * BC : (c + 1) * BC].rearrange("s p b -> p s b"),
            )

        # x streams alone on the SP HWDGE ring, chunk-0 first so compute can
        # start early. wb (7.3 MB) goes on the SWDGE (Pool) ring in parallel;
        # ordering edges below keep output DMAs behind it in the Pool stream.
        for c in range(NB):
            for lo, hi in [(0, 7)] + [(g, min(g + 3, S)) for g in range(7, S, 3)]:
                load_x(lo, hi, c)
        wb_last = None
        for t in range(T):
            wb_last = nc.gpsimd.dma_start(
                out=wt[:, t * 7 * 128 : (t + 1) * 7 * 128].rearrange(
                    "p (m f) -> p m f", m=7
                ),
                in_=wb[t * 7 : (t + 1) * 7].rearrange("m p f -> p m f"),
            )

        for c in range(NB):
            for t in range(T):
                ps = ppool.tile([128, BC], mybir.dt.float32, tag="ps")
                for d in range(7):
                    lhs = wt[:, (t * 7 + d) * 128 : (t * 7 + d + 1) * 128]
                    rhs = xt[:, (t + d) * B + c * BC : (t + d) * B + (c + 1) * BC]
                    nc.tensor.matmul(
                        ps[:], lhs, rhs, start=(d == 0), stop=(d == 6)
                    )
                ot = opool.tile([128, BC], mybir.dt.float32, tag="o")
                nc.scalar.activation(
                    ot[:], ps[:], mybir.ActivationFunctionType.Sigmoid
                )
                od = nc.gpsimd.dma_start(
                    out=y[t, :, c * BC : (c + 1) * BC], in_=ot[:]
                )
                add_dep_helper(
                    od.ins, wb_last.ins, False, "keep outs behind wb on Pool ring"
                )
    nc.compile()
    return nc


def _build_banded(weights: np.ndarray) -> np.ndarray:
    """G[i, d, js, jd] = weight of edge (src row i+d-3, col js) -> (dest row i, col jd).

    dy = 3 - d (dest = src + dy), dx = jd - js, weight index = w[dy+3, dx+3][src_hw].
    """
    w4 = weights.reshape(7, 7, H, W)
    G = np.zeros((H, 7, W, W), np.float32)
    i = np.arange(H)
    for d in range(7):
        r = i + d - 3
        vi = i[(r >= 0) & (r < H)]
        if len(vi) == 0:
            continue
        for dxi in range(7):
            dx = dxi - 3
            js = np.arange(max(0, -dx), W - max(0, dx))
            G[vi[:, None], d, js[None, :], js[None, :] + dx] = w4[6 - d, dxi][
                (vi + d - 3)[:, None], js[None, :]
            ]
    return G


def kernel(x: np.ndarray, weights: np.ndarray) -> np.ndarray:
    global last_exec_time_ns, last_results
    from concourse.bass_utils import run_bass_kernel_spmd

    x = np.ascontiguousarray(x, dtype=np.float32)
    weights = np.ascontiguousarray(weights, dtype=np.float32)

    if MM_MODE not in _cache:
        _cache[MM_MODE] = _build_program(MM_MODE)
    nc = _cache[MM_MODE]

    x3 = x.reshape(H, W, B)
    xp = np.zeros((H + 6, W, B), np.float32)
    xp[3 : H + 3] = x3
    G = _build_banded(weights)

    in_maps = []
    for q in range(NCORES):
        in_maps.append(
            {
                "xs": np.ascontiguousarray(xp[T * q : T * q + S]),
                "wb": np.ascontiguousarray(
                    G[T * q : T * q + T].reshape(T * 7, W, W)
                ),
            }
        )

    trace = os.environ.get("KERNEL_TRACE", "0") == "1"
    res = run_bass_kernel_spmd(
        nc, in_maps, core_ids=list(range(NCORES)), trace=trace
    )
    last_exec_time_ns = res.exec_time_ns
    last_results = res
    out = np.concatenate(
        [r["y"].reshape(T * W, B) for r in res.results], axis=0
    )
    return out


# revision 23
# speedup vs baseline: 1.4309x; 1.4309x over previous
"""Trainium2 Bass kernel for nn_BCNLayer (locally-connected 7x7 lattice layer + sigmoid).

Math: y[i,j,b] = sigmoid( sum_{dy,dx in [-3,3]} w[dy+3,dx+3][(i-dy)*W + (j-dx)]
                          * x[(i-dy)*W + (j-dx), b] )   (zero outside lattice)

Strategy:
  - 8-way shard over lattice rows (H=128 -> 16 dest rows/core, 22 source rows
    with 3-row halos, zero-padded at the edges).
  - For one dest row i and source-row offset d (7 of them), the contribution is
    a banded 128x128 matrix (band +-3 over lattice columns) applied to the
    source row's [128 cols x B batch] slab:  out[jd, b] += sum_js
    Wband[js, jd] * x[js, b].  That is exactly nc.tensor.matmul(psum, lhsT=Wband,
    rhs=xrow) accumulated over the 7 source rows.
  - Banded matrices are prebuilt on the host (numpy) and DMA'd in; HW executes
    pure DMA + matmul + sigmoid.
"""

import os

import numpy as np

H = 128
W = 128
HW = H * W
B = 1024
NCORES = 8
T = H // NCORES  # dest rows per core = 16
S = T + 6        # source rows per core (halo 3 each side) = 22
BC = 512         # batch chunk (fp32 moving-operand max N)
NB = B // BC     # chunks = 2

# dtype mode for the matmul inputs:
#   "f16"  - fp16 x and weights (10-bit mantissa, halves input traffic; fast)
#   "f32r" - tf32 path (10-bit mantissa products, fp32-sized traffic)
#   "f32"  - exact fp32 (4x slower matmul)
MM_MODE = os.environ.get("KERNEL_MM_MODE", "f16")

_cache: dict = {}

# filled by the last kernel() call when KERNEL_TRACE=1
last_exec_time_ns = None
last_results = None


def _build_program(mode: str):
    from contextlib import ExitStack

    import concourse.bacc as bacc
    import concourse.mybir as mybir
    import concourse.tile as tile

    nc = bacc.Bacc(
        "TRN2", target_bir_lowering=False, debug=False, num_devices=NCORES
    )
    mm_dt = {
        "f32": mybir.dt.float32,
        "f32r": mybir.dt.float32r,
        "f16": mybir.dt.float16,
    }[mode]
    xs = nc.dram_tensor("xs", [S, 128, B], mm_dt, kind="ExternalInput").ap()
    wb = nc.dram_tensor(
        "wb", [T * 7, 128, 128], mm_dt, kind="ExternalInput"
    ).ap()
    y = nc.dram_tensor(
        "y", [T, 128, B], mybir.dt.float32, kind="ExternalOutput"
    ).ap()

    from concourse.tile_rust import add_dep_helper

    with tile.TileContext(nc) as tc, ExitStack() as ctx:
        xpool = ctx.enter_context(tc.tile_pool(name="x", bufs=1))
        wpool = ctx.enter_context(tc.tile_pool(name="w", bufs=1))
        ppool = ctx.enter_context(tc.tile_pool(name="ps", bufs=6, space="PSUM"))
        opool = ctx.enter_context(tc.tile_pool(name="o", bufs=6))

        xt = xpool.tile([128, S * B], mm_dt, tag="xslab")
        wt = wpool.tile([128, T * 7 * 128], mm_dt, tag="wslab")

        # Warm the sigmoid ACT table during the load phase (it otherwise loads
        # lazily right before the first real sigmoid, stalling the pipeline).
        warm = opool.tile([128, 1], mybir.dt.float32, tag="warm")
        nc.vector.memset(warm[:], 0.0)
        nc.scalar.activation(warm[:], warm[:], mybir.ActivationFunctionType.Sigmoid)

        # x loads stream on the SP HWDGE ring in batch-chunk-major order so
        # compute (chunk-outer loops below) can start after ~1/4 of the data.
        # wb loads go on the SWDGE (gpsimd) ring, which runs concurrently.
        xt3 = xt[:].rearrange("p (s b) -> p s b", s=S)

        def load_x(lo, hi, c):
            nc.sync.dma_start(
                out=xt3[:, lo:hi, c * BC : (c + 1) * BC],
                in_=xs[lo:hi, :, c * BC : (c + 1) * BC].rearrange("s p b -> p s b"),
            )

        # x streams alone on the SP HWDGE ring, chunk-0 first so compute can
        # start early. wb (7.3 MB) goes on the SWDGE (Pool) ring in parallel;
        # ordering edges below keep output DMAs behind it in the Pool stream.
        for c in range(NB):
            for lo, hi in [(0, 7)] + [(g, min(g + 3, S)) for g in range(7, S, 3)]:
                load_x(lo, hi, c)
        wb_last = None
        for t in range(T):
            wb_last = nc.gpsimd.dma_start(
                out=wt[:, t * 7 * 128 : (t + 1) * 7 * 128].rearrange(
                    "p (m f) -> p m f", m=7
                ),
                in_=wb[t * 7 : (t + 1) * 7].rearrange("m p f -> p m f"),
            )

        for c in range(NB):
            for t in range(T):
                ps = ppool.tile([128, BC], mybir.dt.float32, tag="ps")
                for d in range(7):
                    lhs = wt[:, (t * 7 + d) * 128 : (t * 7 + d + 1) * 128]
                    rhs = xt[:, (t + d) * B + c * BC : (t + d) * B + (c + 1) * BC]
                    nc.tensor.matmul(
                        ps[:], lhs, rhs, start=(d == 0), stop=(d == 6)
                    )
                ot = opool.tile([128, BC], mybir.dt.float32, tag="o")
                nc.scalar.activation(
                    ot[:], ps[:], mybir.ActivationFunctionType.Sigmoid
                )
                od = nc.gpsimd.dma_start(
                    out=y[t, :, c * BC : (c + 1) * BC], in_=ot[:]
                )
                add_dep_helper(
                    od.ins, wb_last.ins, False, "keep outs behind wb on Pool ring"
                )
    nc.compile()
    return nc


def _build_banded(weights: np.ndarray) -> np.ndarray:
    """G[i, d, js, jd] = weight of edge (src row i+d-3, col js) -> (dest row i, col jd).

    dy = 3 - d (dest = src + dy), dx = jd - js, weight index = w[dy+3, dx+3][src_hw].
    """
    w4 = weights.reshape(7, 7, H, W)
    G = np.zeros((H, 7, W, W), np.float32)
    i = np.arange(H)
    for d in range(7):
        r = i + d - 3
        vi = i[(r >= 0) & (r < H)]
        if len(vi) == 0:
            continue
        for dxi in range(7):
            dx = dxi - 3
            js = np.arange(max(0, -dx), W - max(0, dx))
            G[vi[:, None], d, js[None, :], js[None, :] + dx] = w4[6 - d, dxi][
                (vi + d - 3)[:, None], js[None, :]
            ]
    return G


def kernel(x: np.ndarray, weights: np.ndarray) -> np.ndarray:
    global last_exec_time_ns, last_results
    from concourse.bass_utils import run_bass_kernel_spmd

    x = np.ascontiguousarray(x, dtype=np.float32)
    weights = np.ascontiguousarray(weights, dtype=np.float32)

    if MM_MODE not in _cache:
        _cache[MM_MODE] = _build_program(MM_MODE)
    nc = _cache[MM_MODE]

    io_dt = np.float16 if MM_MODE == "f16" else np.float32
    x3 = x.reshape(H, W, B)
    xp = np.zeros((H + 6, W, B), io_dt)
    xp[3 : H + 3] = x3.astype(io_dt)
    G = _build_banded(weights).astype(io_dt)

    in_maps = []
    for q in range(NCORES):
        in_maps.append(
            {
                "xs": np.ascontiguousarray(xp[T * q : T * q + S]),
                "wb": np.ascontiguousarray(
                    G[T * q : T * q + T].reshape(T * 7, W, W)
                ),
            }
        )

    trace = os.environ.get("KERNEL_TRACE", "0") == "1"
    res = run_bass_kernel_spmd(
        nc, in_maps, core_ids=list(range(NCORES)), trace=trace
    )
    last_exec_time_ns = res.exec_time_ns
    last_results = res
    out = np.concatenate(
        [r["y"].reshape(T * W, B) for r in res.results], axis=0
    )
    return out
